# revision 32
# baseline (speedup 1.0000x reference)
"""DSBF (delay-and-sum beamformer) Trainium2 kernel.

Pipeline per core (2 batches/core, data-parallel over 8 cores):
  x --reflect-pad(host)--> xp --chunk-transpose(PE)--> Z[n128, chunks]
  STFT: out[f,t] = sum_q Wq[n,f].T @ Z[:, t+q]  (fp32r matmuls, PSUM accum)
    packed tiles: Re(f0:128), Re(f128:256), Im'(p0=Nyq-re, p1..127=Im f1..127),
    Im(f128:256)  -> exactly 4 stationary tiles, zero waste
  Beamform: per-partition coef vectors, X read straight from PSUM; each
    4-term chain splits into two halves (ACT mul + DVE scalar_tensor_tensor
    each) joined by a GPSIMD tensor-tensor add into the Y buffer.
  ISTFT: OLA fused into PSUM accumulation over shifted Y columns:
    out[r, j] += Bq_kc.T @ Y[:, j-q]  (zero-padded Y; env=1.5 interior
    folded into B; first/last output chunk fixed up by per-partition mul);
    pipelined one t-slice behind the STFT.
  PE-transpose [r,j]->[j,r], wide DMA out (both C duplicates).
Host assembles complex64 y_beamformed and the two Yim rows (f0, f256) that
irfft ignores (from windowed frame dots against xp).
"""

import numpy as np

N_FFT = 512
HOP = 128
F = 257
T = 2501          # frames
PAD = 256
XP_LEN = 320512   # 320000 + 2*PAD
NCHUNK = 2504     # XP_LEN / 128
B_FULL = 16
N_CORES = 8
BL = 2            # batches per core
C = 2
S = 2
LPAD = 8          # Y left pad columns (zeros; keeps 32B-aligned writes)
YW = LPAD + T + 11   # 2520
T_SLICES = [(0, 512), (512, 512), (1024, 512), (1536, 512), (2048, 456)]
J_SLICES = [(2, 512), (514, 512), (1026, 512), (1538, 512), (2050, 452)]

_CONSTS = None
_NC = None
LAST = None  # last BassKernelResults (for test introspection)


def _consts():
    """Host-side constant matrices (float64 math, f32 storage)."""
    global _CONSTS
    if _CONSTS is not None:
        return _CONSTS
    n = np.arange(N_FFT, dtype=np.float64)
    win = 0.5 * (1.0 - np.cos(2.0 * np.pi * n / N_FFT))
    f = np.arange(F, dtype=np.float64)
    ang = 2.0 * np.pi * np.outer(n, f) / N_FFT        # [512, 257]
    cosm = np.cos(ang) * win[:, None]                 # [n, f]
    sinm = -np.sin(ang) * win[:, None]

    # STFT stationaries: wst[q, tile, n128, f128]
    wst = np.zeros((4, 4, 128, 128), np.float64)
    for q in range(4):
        sl = slice(128 * q, 128 * (q + 1))
        wst[q, 0] = cosm[sl, 0:128]
        wst[q, 1] = cosm[sl, 128:256]
        im0 = sinm[sl, 0:128].copy()
        im0[:, 0] = cosm[sl, 256]                     # spare slot: Nyquist re
        wst[q, 2] = im0
        wst[q, 3] = sinm[sl, 128:256]

    # ISTFT stationaries: wist[q, kc, krow128, r128]
    # frames'[t, n] = win[n]/512 * (Yre0 + 2*sum_{1..255}(Yre_f cos - Yim_f sin)
    #                               + Yre256*(-1)^n);  env fold: /1.5
    cf = np.full(F, 2.0)
    cf[0] = 1.0
    cf[256] = 1.0
    bre = (cf[None, :] * np.cos(ang)) * win[:, None] / N_FFT / 1.5   # [n, f]
    bim = (-cf[None, :] * np.sin(ang)) * win[:, None] / N_FFT / 1.5
    wist = np.zeros((4, 4, 128, 128), np.float64)
    for q in range(4):
        sl = slice(128 * q, 128 * (q + 1))
        wist[q, 0] = bre[sl, 0:128].T                 # rows: Yre f0..127
        wist[q, 1] = bre[sl, 128:256].T               # Yre f128..255
        kc2 = bim[sl, 1:128].T                        # Yim f1..127 -> rows 1..127
        blk = np.zeros((128, 128))
        blk[0] = bre[sl, 256]                         # row 0: Yre f256
        blk[1:] = kc2
        wist[q, 2] = blk
        wist[q, 3] = bim[sl, 128:256].T               # Yim f128..255

    # OLA envelope fixups (first & last output chunk)
    n_i = np.arange(N_FFT)
    env = np.zeros(XP_LEN)
    idx = (np.arange(T)[:, None] * HOP + n_i).reshape(-1)
    np.add.at(env, idx, np.tile(win * win, T))
    env_out = env[PAD:XP_LEN - PAD]
    ratio_first = 1.5 / env_out[0:128]
    ratio_last = 1.5 / env_out[-128:]

    ident = np.eye(128, dtype=np.float64)
    _CONSTS = dict(
        win=win,
        wst=wst.astype(np.float32),
        wist=wist.astype(np.float32),
        ratio_first=ratio_first.astype(np.float32),
        ratio_last=ratio_last.astype(np.float32),
        ident=ident.astype(np.float32),
    )
    return _CONSTS


def _coef_pack(wr, wi):
    """Beamform per-partition coefficient vectors -> [128, 66] f32.

    Column layout: idx = ((s*2+g)*2+c)*4 + k, k in {Ar, Br, Ai, Bi}; cols 64/65
    are the env ratio vectors.
      YR[s,g] = sum_c XR[g,c]*Ar + XI'[g,c]*Br
      YI'[s,g] = sum_c XI'[g,c]*Ai + XR[g,c]*Bi
    with the (g=0, p=0) spare-slot exceptions (XI'[0,p0] = Nyquist re).
    """
    cst = _consts()
    coef = np.zeros((128, 66), np.float64)
    for s in range(S):
        for g in range(2):
            fr = slice(128 * g, 128 * (g + 1))
            for c in range(C):
                base = ((s * 2 + g) * 2 + c) * 4
                Ar = wr[s, fr, c].copy()
                Br = wi[s, fr, c].copy()
                Ai = wr[s, fr, c].copy()
                Bi = -wi[s, fr, c].copy()
                if g == 0:
                    Br[0] = 0.0            # XI'[p0] is Nyquist, true XI[f0]=0
                    Ai[0] = wr[s, 256, c]  # YI'[p0] := Yre[f256]
                    Bi[0] = 0.0
                coef[:, base + 0] = Ar
                coef[:, base + 1] = Br
                coef[:, base + 2] = Ai
                coef[:, base + 3] = Bi
    coef[:, 64] = cst["ratio_first"]
    coef[:, 65] = cst["ratio_last"]
    return coef.astype(np.float32)


def _build_nc():
    global _NC
    if _NC is not None:
        return _NC
    import concourse.mybir as mybir
    import concourse.tile as tile
    from concourse import bacc

    f32r = mybir.dt.float32r
    nc = bacc.Bacc(None, target_bir_lowering=False)

    xp = nc.dram_tensor("xp", [BL, C, XP_LEN], f32r, kind="ExternalInput")
    wst_d = nc.dram_tensor("wst", [4, 4, 128, 128], f32r, kind="ExternalInput")
    wist_d = nc.dram_tensor("wist", [4, 4, 128, 128], f32r, kind="ExternalInput")
    coef_d = nc.dram_tensor("coef", [128, 66], mybir.dt.float32, kind="ExternalInput")
    ident_d = nc.dram_tensor("ident", [128, 128], f32r, kind="ExternalInput")

    yt = nc.dram_tensor("yt", [BL, S, C, 2500, 128], f32r, kind="ExternalOutput")
    ybr = nc.dram_tensor("ybr", [BL, S, F, T], f32r, kind="ExternalOutput")
    ybi = nc.dram_tensor("ybi", [BL, S, F, T], f32r, kind="ExternalOutput")

    mul = mybir.AluOpType.mult
    add = mybir.AluOpType.add

    with tile.TileContext(nc) as tc:
        with (
            tc.tile_pool(name="const", bufs=1) as cpool,
            tc.tile_pool(name="ck", bufs=2) as ckpool,
            tc.tile_pool(name="cktail", bufs=1) as ckt_pool,
            tc.tile_pool(name="z", bufs=1) as zpool,
            tc.tile_pool(name="bf", bufs=8) as bfpool,
            tc.tile_pool(name="y", bufs=1) as ypool,
            tc.tile_pool(name="ob", bufs=4) as opool,
            tc.tile_pool(name="ytt", bufs=4) as ytpool,
            tc.tile_pool(name="psmm", bufs=6, space="PSUM") as psmm,
            tc.tile_pool(name="pstr", bufs=2, space="PSUM") as pstr,
        ):
            ident = cpool.tile([128, 128], f32r, tag="ident")
            nc.sync.dma_start(ident[:], ident_d.ap())
            zt = cpool.tile([128, 16], mybir.dt.float32, tag="zt")
            nc.vector.memset(zt[:], 0.0)
            def coef_ap(s, g, c, k):
                return coef_sb[:, ((s * 2 + g) * 2 + c) * 4 + k : ((s * 2 + g) * 2 + c) * 4 + k + 1]

            def zbuild(b):
                """Chunk-transposed input [n128, chunk] per channel."""
                zs = []
                for c in range(C):
                    ck = ckpool.tile([128, 19, 128], f32r, tag="ck")
                    for k0 in range(0, 19, 4):
                        k1 = min(k0 + 4, 19)
                        nc.sync.dma_start(
                            ck[:, k0:k1, :],
                            xp[b, c, k0 * 128 * 128 : k1 * 128 * 128].rearrange(
                                "(k j m) -> j k m", j=128, m=128),
                        )
                    ckt = ckt_pool.tile([72, 128], f32r, tag="ckt")
                    nc.sync.dma_start(
                        ckt[:],
                        xp[b, c, 19 * 128 * 128 : NCHUNK * 128].rearrange("(j m) -> j m", m=128),
                    )
                    z = zpool.tile([128, NCHUNK + 8], f32r, tag=f"z{c}")
                    nc.vector.tensor_copy(z[:, NCHUNK : NCHUNK + 8], zt[:, 0:8])
                    for k in range(19):
                        ps = pstr.tile([128, 128], f32r, tag="pstr")
                        nc.tensor.transpose(ps[:], ck[:, k, :], ident[:])
                        nc.vector.tensor_copy(z[:, 128 * k : 128 * (k + 1)], ps[:])
                    ps = pstr.tile([128, 128], f32r, tag="pstr")
                    nc.tensor.transpose(ps[:, 0:72], ckt[:], ident[0:72, 0:72])
                    nc.vector.tensor_copy(z[:, 2432:2504], ps[:, 0:72])
                    zs.append(z)
                return zs

            zs = zbuild(0)

            wst_sb = cpool.tile([128, 16, 128], f32r, tag="wst")
            nc.scalar.dma_start(wst_sb[:], wst_d.ap().rearrange("q t n f -> n (q t) f"))
            wist_sb = cpool.tile([128, 16, 128], f32r, tag="wist")
            nc.scalar.dma_start(wist_sb[:], wist_d.ap().rearrange("q t n f -> n (q t) f"))
            coef_sb = cpool.tile([128, 66], mybir.dt.float32, tag="coef")
            nc.scalar.dma_start(coef_sb[:], coef_d.ap())

            for b in range(BL):
                # ---- Y buffers with zero pads ----
                ys = [[None] * 4 for _ in range(S)]
                for s in range(S):
                    for tl in range(4):
                        y = ypool.tile([128, YW], f32r, tag=f"y{s}{tl}")
                        nc.vector.tensor_copy(y[:, 0:LPAD], zt[:, 0:LPAD])
                        nc.vector.tensor_copy(y[:, LPAD + T : YW], zt[:, 0 : YW - LPAD - T])
                        ys[s][tl] = y

                def istft_js(s, sl_i):
                    j0, W = J_SLICES[sl_i]
                    ps = psmm.tile([128, 512], mybir.dt.float32, tag="psmm")
                    k = 0
                    for q in range(4):
                        for kc in range(4):
                            nc.tensor.matmul(
                                ps[:, 0:W],
                                wist_sb[:, 4 * q + kc, :],
                                ys[s][kc][:, LPAD + j0 - q : LPAD + j0 - q + W],
                                start=(k == 0),
                                stop=(k == 15),
                            )
                            k += 1
                    ob = opool.tile([128, 512], f32r, tag="ob")
                    nc.scalar.copy(ob[:, 0:W], ps[:, 0:W])
                    if sl_i == 0:
                        nc.vector.tensor_scalar_mul(ob[:, 0:1], ob[:, 0:1], coef_sb[:, 64:65])
                    if sl_i == len(J_SLICES) - 1:
                        nc.vector.tensor_scalar_mul(ob[:, W - 1 : W], ob[:, W - 1 : W], coef_sb[:, 65:66])
                    nkt = (W + 127) // 128
                    ytt = ytpool.tile([128, 4, 128], f32r, tag="ytt")
                    for kt in range(nkt):
                        wj = min(128, W - 128 * kt)
                        pst = pstr.tile([128, 128], f32r, tag="pstr")
                        nc.tensor.transpose(pst[0:wj, :], ob[:, 128 * kt : 128 * kt + wj], ident[:])
                        if kt % 2 == 0:
                            nc.vector.tensor_copy(ytt[0:wj, kt, :], pst[0:wj, :])
                        else:
                            nc.scalar.copy(ytt[0:wj, kt, :], pst[0:wj, :])
                    nfull = W // 128
                    for cdup in range(C):
                        nc.sync.dma_start(
                            yt[b, s, cdup, j0 - 2 : j0 - 2 + 128 * nfull, :].rearrange(
                                "(k j) r -> j k r", j=128),
                            ytt[:, 0:nfull, :],
                        )
                        if nfull < nkt:
                            wj = W - 128 * nfull
                            nc.sync.dma_start(
                                yt[b, s, cdup, j0 - 2 + 128 * nfull : j0 - 2 + W, :],
                                ytt[0:wj, nfull, :],
                            )

                # ---- STFT + beamform per t-slice, ISTFT pipelined one slice behind ----
                # Beamform reads X directly from PSUM; each 4-term chain is
                # split: dst = [ACT mul + DVE stt] + [ACT mul + DVE stt] via a
                # GPSIMD tensor-tensor add.
                for ts_i, (t0, W) in enumerate(T_SLICES):
                    Wx = min(W, T - t0)
                    for g in range(2):
                        pg = {}
                        for c in range(C):
                            for tl in (g, g + 2):
                                ps = psmm.tile([128, 512], mybir.dt.float32, tag="psmm")
                                for q in range(4):
                                    nc.tensor.matmul(
                                        ps[:, 0:W],
                                        wst_sb[:, 4 * q + tl, :],
                                        zs[c][:, t0 + q : t0 + q + W],
                                        start=(q == 0),
                                        stop=(q == 3),
                                    )
                                pg[(c, tl)] = ps
                        chains = []
                        for s in range(S):
                            for kind in range(2):   # 0: YR, 1: YI'
                                if kind == 0:
                                    srcs = (pg[(0, g)], pg[(0, g + 2)], pg[(1, g)], pg[(1, g + 2)])
                                    k0 = 0
                                    dst = ys[s][g][:, LPAD + t0 : LPAD + t0 + Wx]
                                else:
                                    srcs = (pg[(0, g + 2)], pg[(0, g)], pg[(1, g + 2)], pg[(1, g)])
                                    k0 = 2
                                    dst = ys[s][g + 2][:, LPAD + t0 : LPAD + t0 + Wx]
                                tmp = bfpool.tile([128, 512], f32r, tag="bftmp")
                                chains.append((s, k0, srcs, dst, tmp[:, 0:Wx]))
                        # c=0 halves first (frees c=0 psums early for the next group)
                        for s, k0, srcs, dst, tv in chains:
                            nc.scalar.mul(dst, srcs[0][:, 0:Wx], coef_ap(s, g, 0, k0))
                            nc.vector.scalar_tensor_tensor(dst, srcs[1][:, 0:Wx], coef_ap(s, g, 0, k0 + 1), dst, mul, add)
                        for s, k0, srcs, dst, tv in chains:
                            nc.scalar.mul(tv, srcs[2][:, 0:Wx], coef_ap(s, g, 1, k0))
                            nc.vector.scalar_tensor_tensor(tv, srcs[3][:, 0:Wx], coef_ap(s, g, 1, k0 + 1), tv, mul, add)
                        for s, k0, srcs, dst, tv in chains:
                            nc.gpsimd.tensor_tensor(dst, dst, tv, add)
                    if ts_i >= 1:
                        for s in range(S):
                            istft_js(s, ts_i - 1)
                    for s in range(S):
                        # stream y_beamformed out as each t-slice finalizes
                        tsl = slice(LPAD + t0, LPAD + t0 + Wx)
                        osl = slice(t0, t0 + Wx)
                        nc.sync.dma_start(ybr[b, s, 0:128, osl], ys[s][0][:, tsl])
                        nc.sync.dma_start(ybr[b, s, 128:256, osl], ys[s][1][:, tsl])
                        nc.sync.dma_start(ybr[b, s, 256:257, osl], ys[s][2][0:1, tsl])
                        nc.sync.dma_start(ybi[b, s, 1:128, osl], ys[s][2][1:128, tsl])
                        nc.sync.dma_start(ybi[b, s, 128:256, osl], ys[s][3][:, tsl])

                # Next batch's Z build fills the PE gap while the last
                # beamform slice drains on DVE/GP/ACT.
                if b + 1 < BL:
                    zs_next = zbuild(b + 1)

                # ---- last ISTFT slice ----
                for s in range(S):
                    istft_js(s, len(J_SLICES) - 1)

                if b + 1 < BL:
                    zs = zs_next
    nc.compile()
    _NC = nc
    return nc


def kernel(x, steer_re, steer_im):
    from concourse.bass_utils import run_bass_kernel_spmd

    cst = _consts()
    x = np.asarray(x, np.float32)
    sr = np.asarray(steer_re, np.float64)
    si = np.asarray(steer_im, np.float64)

    # MVDR-degenerate weights
    a = sr + 1j * si
    w = a / np.sum(np.abs(a) ** 2, axis=-1, keepdims=True)   # [S, F, C]
    wr, wi = w.real, w.imag
    coef = _coef_pack(wr, wi)

    xp = np.pad(x, [(0, 0), (0, 0), (PAD, PAD)], mode="reflect")

    nc = _build_nc()
    in_maps = []
    for core in range(N_CORES):
        in_maps.append({
            "xp": np.ascontiguousarray(xp[BL * core : BL * (core + 1)]),
            "wst": cst["wst"],
            "wist": cst["wist"],
            "coef": coef,
            "ident": cst["ident"],
        })
    res = run_bass_kernel_spmd(nc, in_maps, core_ids=list(range(N_CORES)))
    global LAST
    LAST = res

    # X rows at f=0 and f=256 (host: windowed frame dots) for the two Yim
    # rows irfft ignores but the y_beamformed output needs.
    win = cst["win"]
    wk = np.stack([win, ((-1.0) ** np.arange(N_FFT)) * win], axis=1)  # [512, 2]
    st = xp.strides
    frames = np.lib.stride_tricks.as_strided(
        xp, shape=(B_FULL, C, T, N_FFT), strides=(st[0], st[1], st[2] * HOP, st[2]))
    xt_full = np.einsum("bctn,nk->bckt", frames, wk.astype(np.float32),
                        dtype=np.float64, casting="unsafe")  # [B, C, 2, T]

    y_time = np.empty((B_FULL, S, C, 320000), np.float32)
    yb = np.empty((B_FULL, S, F, T), np.complex64)
    for core in range(N_CORES):
        r = res.results[core]
        bsl = slice(BL * core, BL * (core + 1))
        y_time[bsl] = r["yt"].reshape(BL, S, C, 320000)
        yb[bsl].real = r["ybr"]
        im = r["ybi"].copy()
        xt = xt_full[bsl]                    # [BL, C, 2, T]
        for s in range(S):
            im[:, s, 0, :] = -(wi[s, 0, 0] * xt[:, 0, 0] + wi[s, 0, 1] * xt[:, 1, 0])
            im[:, s, 256, :] = -(wi[s, 256, 0] * xt[:, 0, 1] + wi[s, 256, 1] * xt[:, 1, 1])
        yb[bsl].imag = im
    return (y_time, yb)
